# revision 15
# baseline (speedup 1.0000x reference)
"""Soft-KNN Bass/Tile kernel for Trainium2 (8 NeuronCores) — v3.

Strategy
--------
- Host prep does ALL data conditioning: per-core shard sorted by label,
  transposed [dim, col] layouts, f32r rounding, y-norm ladder rows,
  query norms, class boundaries (tiled 16x for batched votes).
- Per core, per query-tile (128 queries x 16 tiles): accumulate
  z = X.y - yn into psum per 512-col chunk (bf16 yn-ladder matmul +
  4 f32r product matmuls; the ladder streams at 1 cyc/col and hides in
  the f32r product gaps), scalar-copy psum -> z row halves in SBUF.
- Selection: per half, vector.max (top-8) + max_index. Labels derived
  later from sorted-shard class boundaries.
- AllGather of [2048, 32] f32 (16 z values + 16 idx per query per core),
  3 collectives (qt0-7 / qt8-14 / qt15). Core c owns qtiles {c, c+8}.
- Global phase: merge 128 candidates -> exact top-16 (max8/match_replace
  x2, one-instruction cumsum scan + gpsimd local_scatter compaction),
  d = sqrt(xn - z), softmax, then BATCHED votes: one is_le compare of
  bndt [128,16,101] vs broadcast idx, multiply by broadcast weights,
  tree-reduce over the 16 neighbors (replaces the 16-step ladder).
"""

import ml_dtypes
import numpy as np

import concourse.bass as bass
import concourse.bacc as bacc
import concourse.mybir as mybir
import concourse.tile as tile
from concourse import bass_utils

F32 = mybir.dt.float32
F32R = mybir.dt.float32r
BF16 = mybir.dt.bfloat16
U16 = mybir.dt.uint16
I16 = mybir.dt.int16
AL = mybir.AluOpType
AF = mybir.ActivationFunctionType

NCORES = 8
B = 2048
D = 512
NTRAIN = 50000
COLS = 6400                    # 12*512 + 256; padded equal-boundary shard
CHUNKS = [512] * 12 + [256]
NCHUNK = len(CHUNKS)
QTILES = 16
NCLASS = 100
K = 16                         # global top-k
LK = 16                        # local candidates per core (8 per half-row)
NG = NCORES * LK               # 128 gathered candidates
HSPLIT = 6                     # chunks 0-5 -> half 0; 6-12 -> half 1
H0 = 512 * HSPLIT              # 3072
H1 = COLS - H0                 # 3328
NEG = -3.0e38
VW = NCLASS + 1                # 101 vote boundaries


def _coff(c):
    return sum(CHUNKS[:c])


def build():
    nc = bacc.Bacc("TRN2", target_bir_lowering=False, num_devices=NCORES)

    # ---- dram inputs (host-prepped) ----
    x_in = [nc.dram_tensor(f"x0_{k}", [128, B], F32R, kind="ExternalInput")
            for k in range(4)]
    y_in = [nc.dram_tensor(f"y0_{k}", [128, COLS], F32R, kind="ExternalInput")
            for k in range(4)]
    yn3_in = nc.dram_tensor("yn3", [4, COLS], BF16, kind="ExternalInput")
    xn_in = nc.dram_tensor("xn", [128, QTILES], F32, kind="ExternalInput")
    bndt_in = nc.dram_tensor("bndt", [128, K * VW], F32,
                             kind="ExternalInput")
    out_d = nc.dram_tensor("out", [3 * 128, NCLASS], F32, kind="ExternalOutput")

    ag_in = nc.dram_tensor("ag_in", [B, 2 * LK], F32)
    # 3 collectives: A=qt0-7 (fires mid-run), B=qt8-13 (fires after qt13 so
    # the CC ring is free when the tail needs it), C=qt14+qt15 (fires right
    # after qt15's candidates land; phase 2 serves qt14 on pid 6 and qt15 on
    # pid 7). Pads keep the single-DMA affine reads (base f(pid), stride
    # c->rows-per-core) in range for every pid.
    ag_A = nc.dram_tensor("ag_A", [NCORES * 1024, 2 * LK], F32,
                          addr_space="Shared")
    ag_B = nc.dram_tensor("ag_B", [NCORES * 768 + 896, 2 * LK], F32,
                          addr_space="Shared")
    ag_C = nc.dram_tensor("ag_C", [896 + NCORES * 256 + 128, 2 * LK], F32,
                          addr_space="Shared")

    with tile.TileContext(nc) as tc:
        with tc.tile_pool(name="res", bufs=1) as res, \
             tc.tile_pool(name="zps", bufs=7, space="PSUM") as zps, \
             tc.tile_pool(name="zrowp", bufs=1) as zrowp, \
             tc.tile_pool(name="small", bufs=3) as small:

            # ---- resident tiles (DMA-filled) ----
            xt = [res.tile([128, B], F32R, name=f"xt{k}") for k in range(4)]
            yt = [[res.tile([128, CHUNKS[c]], F32R, name=f"yt{k}_{c}")
                   for c in range(NCHUNK)] for k in range(4)]
            xn_all = res.tile([128, QTILES], F32)
            bndt_f = res.tile([128, K * VW], F32)
            yn3 = res.tile([4, COLS], BF16)
            ones4 = res.tile([4, 128], BF16)

            nc.vector.memset(ones4[:], 0.0)
            nc.vector.memset(ones4[0:3, :], -1.0)
            nc.sync.dma_start(yn3[:], yn3_in[:])

            # critical path first: y chunk 0 + the x columns for qtiles 0-3,
            # then the rest of x/y; ynb/bndt/xn are not needed until the
            # first drains/phases complete, so they go last.
            def _load_y(c):
                co = _coff(c)
                for k in range(4):
                    nc.sync.dma_start(yt[k][c][:],
                                      y_in[k][:, co:co + CHUNKS[c]])
            heng = [nc.scalar, nc.gpsimd, nc.scalar, nc.gpsimd]
            for k in range(4):
                heng[k].dma_start(yt[k][0][:], y_in[k][:, 0:CHUNKS[0]])
                nc.sync.dma_start(xt[k][:, 0:256], x_in[k][:, 0:256])
            for k in range(4):
                heng[k].dma_start(xt[k][:, 256:B], x_in[k][:, 256:B])
            _load_y(1)
            for c in range(2, NCHUNK):
                _load_y(c)
            nc.sync.dma_start(xn_all[:], xn_in[:])
            nc.sync.dma_start(bndt_f[:], bndt_in[:])

            pid_sp = nc.sync.partition_id()
            agA3 = ag_A[:].rearrange("(c r) w -> c r w", c=NCORES)

            def do_qtile(qt):
                qs = qt * 128
                zh = [zrowp.tile([128, H0], F32, name=f"zh0_{qt}", tag="zh0"),
                      zrowp.tile([128, H1], F32, name=f"zh1_{qt}", tag="zh1")]
                for c in range(NCHUNK):
                    cw = CHUNKS[c]
                    co = _coff(c)
                    ps = zps.tile([128, 512], F32)
                    nc.tensor.matmul(ps[:, :cw], ones4[:], yn3[:, co:co + cw],
                                     start=True, stop=False)
                    for k in range(4):
                        nc.tensor.matmul(ps[:, :cw],
                                         xt[k][:, qs:qs + 128],
                                         yt[k][c][:, :cw],
                                         start=False, stop=(k == 3))
                    if c < HSPLIT:
                        nc.scalar.copy(zh[0][:, co:co + cw], ps[:, :cw])
                    else:
                        nc.scalar.copy(zh[1][:, co - H0:co - H0 + cw],
                                       ps[:, :cw])

                cv = small.tile([128, LK], F32, name=f"cv{qt}", tag="cv")
                ci = small.tile([128, LK], U16, name=f"ci{qt}", tag="ci")
                gf = small.tile([128, LK], F32, name=f"gf{qt}", tag="gf")
                for h in range(2):
                    s = slice(h * 8, h * 8 + 8)
                    nc.vector.max(cv[:, s], zh[h][:])
                    nc.vector.max_index(ci[:, s], cv[:, s], zh[h][:])
                nc.vector.tensor_copy(gf[:], ci[:])
                nc.vector.tensor_scalar(out=gf[:, 8:16], in0=gf[:, 8:16],
                                        scalar1=float(H0), scalar2=None,
                                        op0=AL.add)
                nc.sync.dma_start(ag_in[qs:qs + 128, 0:LK], cv[:])
                nc.sync.dma_start(ag_in[qs:qs + 128, LK:2 * LK], gf[:])

            def do_collective(t, lo, hi, pad):
                nc.gpsimd.collective_compute(
                    "AllGather", AL.bypass,
                    replica_groups=[list(range(NCORES))],
                    ins=[ag_in[lo:hi, :].opt()],
                    outs=[t[pad:pad + NCORES * (hi - lo), :].opt()])

            def do_phase(l):
                # l=0: qt=pid from A; l=1: qt=pid+8 from B (junk for pid 7);
                # l=2: qt=15 from C (real for pid 7 only).
                gvl = small.tile([128, NCORES, 2 * LK], F32,
                                 name=f"gvl{l}", tag="gvl", bufs=2)
                if l == 0:
                    src = agA3[:, bass.ds(pid_sp * 128, 128), :]
                elif l == 1:
                    src = (ag_B[bass.ds(pid_sp * 128, NCORES * 768), :]
                           .rearrange("(c r) w -> c r w", c=NCORES)
                           [:, 0:128, :])
                else:
                    # qt14 rows for pid 6 (base 896), qt15 for pid 7 (1024);
                    # pids 0-5 read the junk front pad (output unharvested)
                    src = (ag_C[bass.ds(128 + pid_sp * 128, NCORES * 256), :]
                           .rearrange("(c r) w -> c r w", c=NCORES)
                           [:, 0:128, :])
                nc.sync.dma_start(gvl[:], src.rearrange("c p w -> p c w"))
                gv = gvl[:, :, 0:LK]
                gl = gvl[:, :, LK:2 * LK]

                # exact top-16 of the 128 candidates + payload compaction
                t8a = small.tile([128, 8], F32, name=f"t8a{l}", tag="t8a", bufs=2)
                t8b = small.tile([128, 8], F32, name=f"t8b{l}", tag="t8b", bufs=2)
                m1 = small.tile([128, NG], F32, name=f"m1{l}", tag="m1", bufs=2)
                m2 = small.tile([128, NG], F32, name=f"m2{l}", tag="m2", bufs=2)
                nc.vector.max(t8a[:], gv)
                nc.vector.match_replace(m1[:], t8a[:], gv, NEG)
                nc.vector.max(t8b[:], m1[:])
                nc.vector.match_replace(m2[:], t8b[:], m1[:], NEG)
                mask = small.tile([128, NG], F32, name=f"mk{l}", tag="mk", bufs=2)
                nc.vector.tensor_scalar(out=mask[:], in0=m2[:],
                                        scalar1=-2e38, scalar2=None,
                                        op0=AL.is_le)
                cs = small.tile([128, NG], F32, name=f"cs{l}", tag="cs", bufs=2)
                nc.vector.tensor_tensor_scan(out=cs[:], data0=mask[:],
                                             data1=mask[:], initial=0.0,
                                             op0=AL.add, op1=AL.bypass)
                rk = small.tile([128, NG], F32, name=f"rk{l}", tag="rk", bufs=2)
                nc.vector.tensor_tensor(out=rk[:], in0=cs[:],
                                        in1=mask[:], op=AL.mult)
                nc.vector.tensor_scalar(out=rk[:], in0=rk[:], scalar1=-1.0,
                                        scalar2=None, op0=AL.add)
                rk16 = small.tile([128, NG], I16, name=f"rk16{l}",
                                  tag="rk16", bufs=2)
                nc.vector.tensor_copy(rk16[:], rk[:])

                vlo = small.tile([128, NG], U16, name=f"vlo{l}", tag="vlo", bufs=2)
                vhi = small.tile([128, NG], U16, name=f"vhi{l}", tag="vhi", bufs=2)
                gvu = gv.bitcast(U16).rearrange("p a (b two) -> p a b two",
                                                two=2)
                nc.vector.tensor_copy(vlo[:], gvu[:, :, :, 0:1])
                nc.vector.tensor_copy(vhi[:], gvu[:, :, :, 1:2])
                glu = small.tile([128, NG], U16, name=f"glu{l}", tag="glu", bufs=2)
                nc.vector.tensor_copy(glu[:], gl)
                slo = small.tile([128, K], U16, name=f"slo{l}", tag="slo", bufs=2)
                shi = small.tile([128, K], U16, name=f"shi{l}", tag="shi", bufs=2)
                sla = small.tile([128, K], U16, name=f"sla{l}", tag="sla", bufs=2)
                for plane, out16 in ((vlo, slo), (vhi, shi), (glu, sla)):
                    nc.gpsimd.local_scatter(
                        out16[:].bitcast(I16), plane[:].bitcast(I16),
                        rk16[:], channels=128, num_elems=K, num_idxs=NG)
                v16 = small.tile([128, K], F32, name=f"v16{l}", tag="v16", bufs=2)
                v16u = v16[:].bitcast(U16).rearrange(
                    "p (a two) -> p a two", two=2)
                nc.vector.tensor_copy(v16u[:, :, 0:1], slo[:])
                nc.vector.tensor_copy(v16u[:, :, 1:2], shi[:])
                gidx = small.tile([128, K], F32, name=f"gx{l}", tag="gx", bufs=2)
                nc.vector.tensor_copy(gidx[:], sla[:])

                xn_col = small.tile([128, 1], F32, name=f"xnc{l}", tag="xnc", bufs=2)
                nc.sync.dma_start(
                    xn_col[:],
                    xn_all[:, bass.ds(pid_sp + (8 if l else 0), 1)])
                dsq = small.tile([128, K], F32, name=f"dsq{l}", tag="dsq", bufs=2)
                nc.scalar.activation(dsq[:], v16[:], AF.Sqrt, scale=-1.0,
                                     bias=xn_col[:, 0:1])
                ew = small.tile([128, K], F32, name=f"ew{l}", tag="ew", bufs=2)
                zsum = small.tile([128, 1], F32, name=f"zs{l}", tag="zs", bufs=2)
                nc.scalar.activation(ew[:], dsq[:], AF.Exp, scale=-1.0,
                                     accum_out=zsum[:, 0:1])
                rz = small.tile([128, 1], F32, name=f"rz{l}", tag="rz", bufs=2)
                nc.vector.reciprocal(rz[:], zsum[:])
                wt = small.tile([128, K], F32, name=f"wt{l}", tag="wt", bufs=2)
                nc.vector.tensor_scalar(out=wt[:], in0=ew[:],
                                        scalar1=rz[:, 0:1], scalar2=None,
                                        op0=AL.mult)
                # batched votes: S[q, r, c] = w_r * [bnd_c <= idx_r],
                # tree-reduced over r; vote_c = S_c - S_{c+1}.
                bnd3 = bndt_f[:].rearrange("p (r c) -> p r c", r=K)
                gidx_b = gidx[:].unsqueeze(2).broadcast_to([128, K, VW])
                wt_b = wt[:].unsqueeze(2).broadcast_to([128, K, VW])
                sm = small.tile([128, K, VW], F32, name=f"sm{l}", tag="sm", bufs=1)
                nc.vector.tensor_tensor(out=sm[:], in0=bnd3, in1=gidx_b,
                                        op=AL.is_le)
                nc.vector.tensor_tensor(out=sm[:], in0=sm[:], in1=wt_b,
                                        op=AL.mult)
                for half in (8, 4, 2, 1):
                    nc.vector.tensor_tensor(
                        out=sm[:, 0:half, :], in0=sm[:, 0:half, :],
                        in1=sm[:, half:2 * half, :], op=AL.add)
                vote = small.tile([128, NCLASS], F32, name=f"vote{l}",
                                  tag="vote", bufs=2)
                nc.vector.tensor_tensor(out=vote[:],
                                        in0=sm[:, 0, 0:NCLASS],
                                        in1=sm[:, 0, 1:VW],
                                        op=AL.subtract)
                nc.sync.dma_start(out_d[l * 128:(l + 1) * 128, :], vote[:])

            for qt in range(8):
                do_qtile(qt)
            do_collective(ag_A, 0, 1024, 0)
            do_qtile(8)
            do_phase(0)
            for qt in range(9, 14):
                do_qtile(qt)
            do_collective(ag_B, 1024, 1792, 0)
            do_qtile(14)
            do_qtile(15)
            do_phase(1)
            do_collective(ag_C, 1792, 2048, 896)
            do_phase(2)

    nc.finalize()
    return nc


def _bf16(a):
    """Round fp32 -> bf16 (round-to-nearest-even), keep fp32 container."""
    u = a.view(np.uint32)
    rounded = (u.astype(np.uint64) + 0x7FFF
               + ((u >> 16) & 1)) >> 16
    return (rounded.astype(np.uint32) << 16).view(np.float32)


def _f32r(a):
    i = a.view(np.uint32).astype(np.int64)
    i = (i + 0x400) & ~0x7FF
    return (i & 0xFFFFFFFF).astype(np.uint32).view(np.float32)


def _bf16_bits(a):
    """fp32 -> bf16 (ml_dtypes.bfloat16 array, round-to-nearest-even)."""
    u = a.view(np.uint32)
    rounded = (u.astype(np.uint64) + 0x7FFF + ((u >> 16) & 1)) >> 16
    return rounded.astype(np.uint16).view(ml_dtypes.bfloat16)


def _host_prep(x, tf, tl):
    """Produce per-core input dicts."""
    x = np.ascontiguousarray(np.asarray(x, np.float32))
    tf = np.ascontiguousarray(np.asarray(tf, np.float32))
    tl = np.asarray(tl, np.int64)

    X = 2.0 * x                                    # fold z = X.y - yn
    xn = np.sum(x.astype(np.float64) * x, axis=1).astype(np.float32)
    xn_all = xn.reshape(QTILES, 128).T.copy()      # [128, 16]

    xparts = [_f32r(X)]

    # equal-boundary deal: global sort by label, class c dealt round-robin
    # to cores, padded so every core has identical class boundaries.
    perm = np.argsort(tl, kind="stable")
    tf_s = tf[perm]
    counts = np.bincount(tl, minlength=NCLASS)
    m = (counts + NCORES - 1) // NCORES            # per-core padded counts
    assert m.sum() <= COLS, m.sum()
    bnd = np.concatenate([[0], np.cumsum(m)[:-1]])
    bnd_ext = np.concatenate([bnd.astype(np.float32) - 0.5, [1.0e9]])
    bndt = np.tile(bnd_ext.astype(np.float32), K)
    bndt_b = np.broadcast_to(bndt, (128, K * (NCLASS + 1))).copy()
    gstart = np.concatenate([[0], np.cumsum(counts)[:-1]])

    core_feats = np.zeros((NCORES, COLS, D), np.float32)
    core_valid = np.zeros((NCORES, COLS), bool)
    for cls in range(NCLASS):
        rows = tf_s[gstart[cls]:gstart[cls] + counts[cls]]
        for c in range(NCORES):
            part = rows[c::NCORES]
            core_feats[c, bnd[cls]:bnd[cls] + len(part)] = part
            core_valid[c, bnd[cls]:bnd[cls] + len(part)] = True

    in_maps = []
    for c in range(NCORES):
        feats = core_feats[c]
        yn = np.sum(feats.astype(np.float64) * feats,
                    axis=1).astype(np.float32)
        ynp = np.where(core_valid[c], yn, np.float32(1.0e30))
        y1 = _bf16(ynp); r1 = ynp - y1
        y2 = _bf16(r1); r2 = r1 - y2
        y3 = _bf16(r2)
        yn3 = np.zeros((4, COLS), ml_dtypes.bfloat16)
        yn3[0] = _bf16_bits(y1)
        yn3[1] = _bf16_bits(y2)
        yn3[2] = _bf16_bits(y3)

        yparts = [_f32r(feats)]

        mm = {
            "yn3": yn3,
            "xn": xn_all,
            "bndt": bndt_b,
        }
        for p, xp in enumerate(xparts):
            # [2048, 512] -> k blocks [128, 2048]
            for k in range(4):
                blk = np.ascontiguousarray(xp[:, k * 128:(k + 1) * 128].T)
                mm[f"x{p}_{k}"] = blk
        for p, yp in enumerate(yparts):
            for k in range(4):
                blk = np.ascontiguousarray(yp[:, k * 128:(k + 1) * 128].T)
                mm[f"y{p}_{k}"] = blk
        in_maps.append(mm)
    return in_maps


_NC_CACHE = {}
LAST_RESULTS = None


def kernel(x, train_features, train_labels, **run_kwargs):
    global LAST_RESULTS
    in_maps = _host_prep(x, train_features, train_labels)
    if "v3" not in _NC_CACHE:
        _NC_CACHE["v3"] = build()
    res = bass_utils.run_bass_kernel_spmd(
        _NC_CACHE["v3"], in_maps, core_ids=list(range(NCORES)),
        **run_kwargs)
    LAST_RESULTS = res
    out = np.zeros((B, NCLASS), np.float32)
    for c in range(NCORES):
        o = res.results[c]["out"]
        out[c * 128:(c + 1) * 128] = o[0:128]
        blk = 1 if c < 6 else 2
        out[(8 + c) * 128:(9 + c) * 128] = o[blk * 128:(blk + 1) * 128]
    return out


# revision 16
# speedup vs baseline: 1.0224x; 1.0224x over previous
"""Soft-KNN Bass/Tile kernel for Trainium2 (8 NeuronCores) — v3.

Strategy
--------
- Host prep does ALL data conditioning: per-core shard sorted by label,
  transposed [dim, col] layouts, f32r rounding, y-norm ladder rows,
  query norms, class boundaries (tiled 16x for batched votes).
- Per core, per query-tile (128 queries x 16 tiles): accumulate
  z = X.y - yn into psum per 512-col chunk (bf16 yn-ladder matmul +
  4 f32r product matmuls; the ladder streams at 1 cyc/col and hides in
  the f32r product gaps), scalar-copy psum -> z row halves in SBUF.
- Selection: per half, vector.max (top-8) + max_index. Labels derived
  later from sorted-shard class boundaries.
- AllGather of [2048, 32] f32 (16 z values + 16 idx per query per core),
  3 collectives (qt0-7 / qt8-14 / qt15). Core c owns qtiles {c, c+8}.
- Global phase: merge 128 candidates -> exact top-16 (max8/match_replace
  x2, one-instruction cumsum scan + gpsimd local_scatter compaction),
  d = sqrt(xn - z), softmax, then BATCHED votes: one is_le compare of
  bndt [128,16,101] vs broadcast idx, multiply by broadcast weights,
  tree-reduce over the 16 neighbors (replaces the 16-step ladder).
"""

import ml_dtypes
import numpy as np

import concourse.bass as bass
import concourse.bacc as bacc
import concourse.mybir as mybir
import concourse.tile as tile
from concourse import bass_utils

F32 = mybir.dt.float32
F32R = mybir.dt.float32r
BF16 = mybir.dt.bfloat16
U16 = mybir.dt.uint16
I16 = mybir.dt.int16
AL = mybir.AluOpType
AF = mybir.ActivationFunctionType

NCORES = 8
B = 2048
D = 512
NTRAIN = 50000
COLS = 6400                    # 12*512 + 256; padded equal-boundary shard
CHUNKS = [512] * 12 + [256]
NCHUNK = len(CHUNKS)
QTILES = 16
NCLASS = 100
K = 16                         # global top-k
LK = 16                        # local candidates per core (8 per half-row)
NG = NCORES * LK               # 128 gathered candidates
HSPLIT = 6                     # chunks 0-5 -> half 0; 6-12 -> half 1
H0 = 512 * HSPLIT              # 3072
H1 = COLS - H0                 # 3328
NEG = -3.0e38
VW = NCLASS + 1                # 101 vote boundaries


def _coff(c):
    return sum(CHUNKS[:c])


def build():
    nc = bacc.Bacc("TRN2", target_bir_lowering=False, num_devices=NCORES)

    # ---- dram inputs (host-prepped) ----
    x_in = [nc.dram_tensor(f"x0_{k}", [128, B], F32R, kind="ExternalInput")
            for k in range(4)]
    y_in = [nc.dram_tensor(f"y0_{k}", [128, COLS], F32R, kind="ExternalInput")
            for k in range(4)]
    yn3_in = nc.dram_tensor("yn3", [4, COLS], BF16, kind="ExternalInput")
    xn_in = nc.dram_tensor("xn", [128, QTILES], F32, kind="ExternalInput")
    bndt_in = nc.dram_tensor("bndt", [128, K * VW], F32,
                             kind="ExternalInput")
    out_d = nc.dram_tensor("out", [3 * 128, NCLASS], F32, kind="ExternalOutput")

    ag_in = nc.dram_tensor("ag_in", [B, 2 * LK], F32)
    # 3 collectives: A=qt0-7 (fires mid-run), B=qt8-13 (fires after qt13 so
    # the CC ring is free when the tail needs it), C=qt14+qt15 (fires right
    # after qt15's candidates land; phase 2 serves qt14 on pid 6 and qt15 on
    # pid 7). Pads keep the single-DMA affine reads (base f(pid), stride
    # c->rows-per-core) in range for every pid.
    ag_A = nc.dram_tensor("ag_A", [NCORES * 1024, 2 * LK], F32,
                          addr_space="Shared")
    ag_B = nc.dram_tensor("ag_B", [NCORES * 768 + 896, 2 * LK], F32,
                          addr_space="Shared")
    ag_C = nc.dram_tensor("ag_C", [896 + NCORES * 256 + 128, 2 * LK], F32,
                          addr_space="Shared")

    with tile.TileContext(nc) as tc:
        with tc.tile_pool(name="res", bufs=1) as res, \
             tc.tile_pool(name="zps", bufs=7, space="PSUM") as zps, \
             tc.tile_pool(name="zrowp", bufs=1) as zrowp, \
             tc.tile_pool(name="small", bufs=3) as small:

            # ---- resident tiles (DMA-filled) ----
            xt = [res.tile([128, B], F32R, name=f"xt{k}") for k in range(4)]
            yt = [[res.tile([128, CHUNKS[c]], F32R, name=f"yt{k}_{c}")
                   for c in range(NCHUNK)] for k in range(4)]
            xn_all = res.tile([128, QTILES], F32)
            bndt_f = res.tile([128, K * VW], F32)
            yn3 = res.tile([4, COLS], BF16)
            ones4 = res.tile([4, 128], BF16)

            nc.vector.memset(ones4[:], 0.0)
            nc.vector.memset(ones4[0:3, :], -1.0)
            nc.sync.dma_start(yn3[:], yn3_in[:])

            # critical path first: y chunk 0 + the x columns for qtiles 0-3,
            # then the rest of x/y; ynb/bndt/xn are not needed until the
            # first drains/phases complete, so they go last.
            def _load_y(c):
                co = _coff(c)
                for k in range(4):
                    nc.sync.dma_start(yt[k][c][:],
                                      y_in[k][:, co:co + CHUNKS[c]])
            _load_y(0)
            for k in range(4):
                nc.sync.dma_start(xt[k][:, 0:512], x_in[k][:, 0:512])
            _load_y(1)
            for k in range(4):
                nc.sync.dma_start(xt[k][:, 512:B], x_in[k][:, 512:B])
            for c in range(2, NCHUNK):
                _load_y(c)
            nc.sync.dma_start(xn_all[:], xn_in[:])
            nc.sync.dma_start(bndt_f[:], bndt_in[:])

            pid_sp = nc.sync.partition_id()
            agA3 = ag_A[:].rearrange("(c r) w -> c r w", c=NCORES)

            def do_qtile(qt):
                qs = qt * 128
                zh = [zrowp.tile([128, H0], F32, name=f"zh0_{qt}", tag="zh0"),
                      zrowp.tile([128, H1], F32, name=f"zh1_{qt}", tag="zh1")]
                for c in range(NCHUNK):
                    cw = CHUNKS[c]
                    co = _coff(c)
                    ps = zps.tile([128, 512], F32)
                    nc.tensor.matmul(ps[:, :cw], ones4[:], yn3[:, co:co + cw],
                                     start=True, stop=False)
                    for k in range(4):
                        nc.tensor.matmul(ps[:, :cw],
                                         xt[k][:, qs:qs + 128],
                                         yt[k][c][:, :cw],
                                         start=False, stop=(k == 3))
                    if c < HSPLIT:
                        nc.scalar.copy(zh[0][:, co:co + cw], ps[:, :cw])
                    else:
                        nc.scalar.copy(zh[1][:, co - H0:co - H0 + cw],
                                       ps[:, :cw])

                cv = small.tile([128, LK], F32, name=f"cv{qt}", tag="cv")
                ci = small.tile([128, LK], U16, name=f"ci{qt}", tag="ci")
                gf = small.tile([128, LK], F32, name=f"gf{qt}", tag="gf")
                for h in range(2):
                    s = slice(h * 8, h * 8 + 8)
                    nc.vector.max(cv[:, s], zh[h][:])
                    nc.vector.max_index(ci[:, s], cv[:, s], zh[h][:])
                nc.vector.tensor_copy(gf[:], ci[:])
                nc.vector.tensor_scalar(out=gf[:, 8:16], in0=gf[:, 8:16],
                                        scalar1=float(H0), scalar2=None,
                                        op0=AL.add)
                nc.sync.dma_start(ag_in[qs:qs + 128, 0:LK], cv[:])
                nc.sync.dma_start(ag_in[qs:qs + 128, LK:2 * LK], gf[:])

            def do_collective(t, lo, hi, pad):
                nc.gpsimd.collective_compute(
                    "AllGather", AL.bypass,
                    replica_groups=[list(range(NCORES))],
                    ins=[ag_in[lo:hi, :].opt()],
                    outs=[t[pad:pad + NCORES * (hi - lo), :].opt()])

            def do_phase(l):
                # l=0: qt=pid from A; l=1: qt=pid+8 from B (junk for pid 7);
                # l=2: qt=15 from C (real for pid 7 only).
                gvl = small.tile([128, NCORES, 2 * LK], F32,
                                 name=f"gvl{l}", tag="gvl", bufs=2)
                if l == 0:
                    src = agA3[:, bass.ds(pid_sp * 128, 128), :]
                elif l == 1:
                    src = (ag_B[bass.ds(pid_sp * 128, NCORES * 768), :]
                           .rearrange("(c r) w -> c r w", c=NCORES)
                           [:, 0:128, :])
                else:
                    # qt14 rows for pid 6 (base 896), qt15 for pid 7 (1024);
                    # pids 0-5 read the junk front pad (output unharvested)
                    src = (ag_C[bass.ds(128 + pid_sp * 128, NCORES * 256), :]
                           .rearrange("(c r) w -> c r w", c=NCORES)
                           [:, 0:128, :])
                nc.sync.dma_start(gvl[:], src.rearrange("c p w -> p c w"))
                gv = gvl[:, :, 0:LK]
                gl = gvl[:, :, LK:2 * LK]

                # exact top-16 of the 128 candidates + payload compaction
                t8a = small.tile([128, 8], F32, name=f"t8a{l}", tag="t8a", bufs=2)
                t8b = small.tile([128, 8], F32, name=f"t8b{l}", tag="t8b", bufs=2)
                m1 = small.tile([128, NG], F32, name=f"m1{l}", tag="m1", bufs=2)
                m2 = small.tile([128, NG], F32, name=f"m2{l}", tag="m2", bufs=2)
                nc.vector.max(t8a[:], gv)
                nc.vector.match_replace(m1[:], t8a[:], gv, NEG)
                nc.vector.max(t8b[:], m1[:])
                nc.vector.match_replace(m2[:], t8b[:], m1[:], NEG)
                mask = small.tile([128, NG], F32, name=f"mk{l}", tag="mk", bufs=2)
                nc.vector.tensor_scalar(out=mask[:], in0=m2[:],
                                        scalar1=-2e38, scalar2=None,
                                        op0=AL.is_le)
                cs = small.tile([128, NG], F32, name=f"cs{l}", tag="cs", bufs=2)
                nc.vector.tensor_tensor_scan(out=cs[:], data0=mask[:],
                                             data1=mask[:], initial=0.0,
                                             op0=AL.add, op1=AL.bypass)
                rk = small.tile([128, NG], F32, name=f"rk{l}", tag="rk", bufs=2)
                nc.vector.tensor_tensor(out=rk[:], in0=cs[:],
                                        in1=mask[:], op=AL.mult)
                nc.vector.tensor_scalar(out=rk[:], in0=rk[:], scalar1=-1.0,
                                        scalar2=None, op0=AL.add)
                rk16 = small.tile([128, NG], I16, name=f"rk16{l}",
                                  tag="rk16", bufs=2)
                nc.vector.tensor_copy(rk16[:], rk[:])

                vlo = small.tile([128, NG], U16, name=f"vlo{l}", tag="vlo", bufs=2)
                vhi = small.tile([128, NG], U16, name=f"vhi{l}", tag="vhi", bufs=2)
                gvu = gv.bitcast(U16).rearrange("p a (b two) -> p a b two",
                                                two=2)
                nc.vector.tensor_copy(vlo[:], gvu[:, :, :, 0:1])
                nc.vector.tensor_copy(vhi[:], gvu[:, :, :, 1:2])
                glu = small.tile([128, NG], U16, name=f"glu{l}", tag="glu", bufs=2)
                nc.vector.tensor_copy(glu[:], gl)
                slo = small.tile([128, K], U16, name=f"slo{l}", tag="slo", bufs=2)
                shi = small.tile([128, K], U16, name=f"shi{l}", tag="shi", bufs=2)
                sla = small.tile([128, K], U16, name=f"sla{l}", tag="sla", bufs=2)
                for plane, out16 in ((vlo, slo), (vhi, shi), (glu, sla)):
                    nc.gpsimd.local_scatter(
                        out16[:].bitcast(I16), plane[:].bitcast(I16),
                        rk16[:], channels=128, num_elems=K, num_idxs=NG)
                v16 = small.tile([128, K], F32, name=f"v16{l}", tag="v16", bufs=2)
                v16u = v16[:].bitcast(U16).rearrange(
                    "p (a two) -> p a two", two=2)
                nc.vector.tensor_copy(v16u[:, :, 0:1], slo[:])
                nc.vector.tensor_copy(v16u[:, :, 1:2], shi[:])
                gidx = small.tile([128, K], F32, name=f"gx{l}", tag="gx", bufs=2)
                nc.vector.tensor_copy(gidx[:], sla[:])

                xn_col = small.tile([128, 1], F32, name=f"xnc{l}", tag="xnc", bufs=2)
                nc.sync.dma_start(
                    xn_col[:],
                    xn_all[:, bass.ds(pid_sp + (8 if l else 0), 1)])
                dsq = small.tile([128, K], F32, name=f"dsq{l}", tag="dsq", bufs=2)
                nc.scalar.activation(dsq[:], v16[:], AF.Sqrt, scale=-1.0,
                                     bias=xn_col[:, 0:1])
                ew = small.tile([128, K], F32, name=f"ew{l}", tag="ew", bufs=2)
                zsum = small.tile([128, 1], F32, name=f"zs{l}", tag="zs", bufs=2)
                nc.scalar.activation(ew[:], dsq[:], AF.Exp, scale=-1.0,
                                     accum_out=zsum[:, 0:1])
                rz = small.tile([128, 1], F32, name=f"rz{l}", tag="rz", bufs=2)
                nc.vector.reciprocal(rz[:], zsum[:])
                wt = small.tile([128, K], F32, name=f"wt{l}", tag="wt", bufs=2)
                nc.vector.tensor_scalar(out=wt[:], in0=ew[:],
                                        scalar1=rz[:, 0:1], scalar2=None,
                                        op0=AL.mult)
                # batched votes: S[q, r, c] = w_r * [bnd_c <= idx_r],
                # tree-reduced over r; vote_c = S_c - S_{c+1}.
                bnd3 = bndt_f[:].rearrange("p (r c) -> p r c", r=K)
                gidx_b = gidx[:].unsqueeze(2).broadcast_to([128, K, VW])
                wt_b = wt[:].unsqueeze(2).broadcast_to([128, K, VW])
                sm = small.tile([128, K, VW], F32, name=f"sm{l}", tag="sm", bufs=1)
                nc.vector.tensor_tensor(out=sm[:], in0=bnd3, in1=gidx_b,
                                        op=AL.is_le)
                nc.vector.tensor_tensor(out=sm[:], in0=sm[:], in1=wt_b,
                                        op=AL.mult)
                for half in (8, 4, 2, 1):
                    nc.vector.tensor_tensor(
                        out=sm[:, 0:half, :], in0=sm[:, 0:half, :],
                        in1=sm[:, half:2 * half, :], op=AL.add)
                vote = small.tile([128, NCLASS], F32, name=f"vote{l}",
                                  tag="vote", bufs=2)
                nc.vector.tensor_tensor(out=vote[:],
                                        in0=sm[:, 0, 0:NCLASS],
                                        in1=sm[:, 0, 1:VW],
                                        op=AL.subtract)
                nc.sync.dma_start(out_d[l * 128:(l + 1) * 128, :], vote[:])

            for qt in range(8):
                do_qtile(qt)
            do_collective(ag_A, 0, 1024, 0)
            do_qtile(8)
            do_phase(0)
            for qt in range(9, 14):
                do_qtile(qt)
            do_collective(ag_B, 1024, 1792, 0)
            do_qtile(14)
            do_qtile(15)
            do_phase(1)
            do_collective(ag_C, 1792, 2048, 896)
            do_phase(2)

    nc.finalize()
    return nc


def _bf16(a):
    """Round fp32 -> bf16 (round-to-nearest-even), keep fp32 container."""
    u = a.view(np.uint32)
    rounded = (u.astype(np.uint64) + 0x7FFF
               + ((u >> 16) & 1)) >> 16
    return (rounded.astype(np.uint32) << 16).view(np.float32)


def _f32r(a):
    i = a.view(np.uint32).astype(np.int64)
    i = (i + 0x400) & ~0x7FF
    return (i & 0xFFFFFFFF).astype(np.uint32).view(np.float32)


def _bf16_bits(a):
    """fp32 -> bf16 (ml_dtypes.bfloat16 array, round-to-nearest-even)."""
    u = a.view(np.uint32)
    rounded = (u.astype(np.uint64) + 0x7FFF + ((u >> 16) & 1)) >> 16
    return rounded.astype(np.uint16).view(ml_dtypes.bfloat16)


def _host_prep(x, tf, tl):
    """Produce per-core input dicts."""
    x = np.ascontiguousarray(np.asarray(x, np.float32))
    tf = np.ascontiguousarray(np.asarray(tf, np.float32))
    tl = np.asarray(tl, np.int64)

    X = 2.0 * x                                    # fold z = X.y - yn
    xn = np.sum(x.astype(np.float64) * x, axis=1).astype(np.float32)
    xn_all = xn.reshape(QTILES, 128).T.copy()      # [128, 16]

    xparts = [_f32r(X)]

    # equal-boundary deal: global sort by label, class c dealt round-robin
    # to cores, padded so every core has identical class boundaries.
    perm = np.argsort(tl, kind="stable")
    tf_s = tf[perm]
    counts = np.bincount(tl, minlength=NCLASS)
    m = (counts + NCORES - 1) // NCORES            # per-core padded counts
    assert m.sum() <= COLS, m.sum()
    bnd = np.concatenate([[0], np.cumsum(m)[:-1]])
    bnd_ext = np.concatenate([bnd.astype(np.float32) - 0.5, [1.0e9]])
    bndt = np.tile(bnd_ext.astype(np.float32), K)
    bndt_b = np.broadcast_to(bndt, (128, K * (NCLASS + 1))).copy()
    gstart = np.concatenate([[0], np.cumsum(counts)[:-1]])

    core_feats = np.zeros((NCORES, COLS, D), np.float32)
    core_valid = np.zeros((NCORES, COLS), bool)
    for cls in range(NCLASS):
        rows = tf_s[gstart[cls]:gstart[cls] + counts[cls]]
        for c in range(NCORES):
            part = rows[c::NCORES]
            core_feats[c, bnd[cls]:bnd[cls] + len(part)] = part
            core_valid[c, bnd[cls]:bnd[cls] + len(part)] = True

    in_maps = []
    for c in range(NCORES):
        feats = core_feats[c]
        yn = np.sum(feats.astype(np.float64) * feats,
                    axis=1).astype(np.float32)
        ynp = np.where(core_valid[c], yn, np.float32(1.0e30))
        y1 = _bf16(ynp); r1 = ynp - y1
        y2 = _bf16(r1); r2 = r1 - y2
        y3 = _bf16(r2)
        yn3 = np.zeros((4, COLS), ml_dtypes.bfloat16)
        yn3[0] = _bf16_bits(y1)
        yn3[1] = _bf16_bits(y2)
        yn3[2] = _bf16_bits(y3)

        yparts = [_f32r(feats)]

        mm = {
            "yn3": yn3,
            "xn": xn_all,
            "bndt": bndt_b,
        }
        for p, xp in enumerate(xparts):
            # [2048, 512] -> k blocks [128, 2048]
            for k in range(4):
                blk = np.ascontiguousarray(xp[:, k * 128:(k + 1) * 128].T)
                mm[f"x{p}_{k}"] = blk
        for p, yp in enumerate(yparts):
            for k in range(4):
                blk = np.ascontiguousarray(yp[:, k * 128:(k + 1) * 128].T)
                mm[f"y{p}_{k}"] = blk
        in_maps.append(mm)
    return in_maps


_NC_CACHE = {}
LAST_RESULTS = None


def kernel(x, train_features, train_labels, **run_kwargs):
    global LAST_RESULTS
    in_maps = _host_prep(x, train_features, train_labels)
    if "v3" not in _NC_CACHE:
        _NC_CACHE["v3"] = build()
    res = bass_utils.run_bass_kernel_spmd(
        _NC_CACHE["v3"], in_maps, core_ids=list(range(NCORES)),
        **run_kwargs)
    LAST_RESULTS = res
    out = np.zeros((B, NCLASS), np.float32)
    for c in range(NCORES):
        o = res.results[c]["out"]
        out[c * 128:(c + 1) * 128] = o[0:128]
        blk = 1 if c < 6 else 2
        out[(8 + c) * 128:(9 + c) * 128] = o[blk * 128:(blk + 1) * 128]
    return out


# revision 19
# speedup vs baseline: 1.0276x; 1.0050x over previous
"""Soft-KNN Bass/Tile kernel for Trainium2 (8 NeuronCores) — v3.

Strategy
--------
- Host prep does ALL data conditioning: per-core shard sorted by label,
  transposed [dim, col] layouts, f32r rounding, y-norm ladder rows,
  query norms, class boundaries (tiled 16x for batched votes).
- Per core, per query-tile (128 queries x 16 tiles): accumulate
  z = X.y - yn into psum per 512-col chunk (bf16 yn-ladder matmul +
  4 f32r product matmuls; the ladder streams at 1 cyc/col and hides in
  the f32r product gaps), scalar-copy psum -> z row halves in SBUF.
- Selection: per half, vector.max (top-8) + max_index. Labels derived
  later from sorted-shard class boundaries.
- AllGather of [2048, 32] f32 (16 z values + 16 idx per query per core),
  3 collectives (qt0-7 / qt8-14 / qt15). Core c owns qtiles {c, c+8}.
- Global phase: merge 128 candidates -> exact top-16 (max8/match_replace
  x2, one-instruction cumsum scan + gpsimd local_scatter compaction),
  d = sqrt(xn - z), softmax, then BATCHED votes: one is_le compare of
  bndt [128,16,101] vs broadcast idx, multiply by broadcast weights,
  tree-reduce over the 16 neighbors (replaces the 16-step ladder).
"""

import ml_dtypes
import numpy as np

import concourse.bass as bass
import concourse.bacc as bacc
import concourse.mybir as mybir
import concourse.tile as tile
from concourse import bass_utils

F32 = mybir.dt.float32
F32R = mybir.dt.float32r
BF16 = mybir.dt.bfloat16
U16 = mybir.dt.uint16
I16 = mybir.dt.int16
AL = mybir.AluOpType
AF = mybir.ActivationFunctionType

NCORES = 8
B = 2048
D = 512
NTRAIN = 50000
COLS = 6304                    # 12*512 + 160; padded equal-boundary shard
CHUNKS = [512] * 12 + [160]
NCHUNK = len(CHUNKS)
QTILES = 16
NCLASS = 100
K = 16                         # global top-k
LK = 16                        # local candidates per core (8 per half-row)
NG = NCORES * LK               # 128 gathered candidates
HSPLIT = 6                     # chunks 0-5 -> half 0; 6-12 -> half 1
H0 = 512 * HSPLIT              # 3072
H1 = COLS - H0                 # 3328
NEG = -3.0e38
VW = NCLASS + 1                # 101 vote boundaries


def _coff(c):
    return sum(CHUNKS[:c])


def build():
    nc = bacc.Bacc("TRN2", target_bir_lowering=False, num_devices=NCORES)

    # ---- dram inputs (host-prepped) ----
    x_in = [nc.dram_tensor(f"x0_{k}", [128, B], F32R, kind="ExternalInput")
            for k in range(4)]
    y_in = [nc.dram_tensor(f"y0_{k}", [128, COLS], F32R, kind="ExternalInput")
            for k in range(4)]
    yn3_in = nc.dram_tensor("yn3", [4, COLS], BF16, kind="ExternalInput")
    xn_in = nc.dram_tensor("xn", [128, QTILES], F32, kind="ExternalInput")
    bndt_in = nc.dram_tensor("bndt", [128, K * VW], F32,
                             kind="ExternalInput")
    out_d = nc.dram_tensor("out", [3 * 128, NCLASS], F32, kind="ExternalOutput")

    ag_in = nc.dram_tensor("ag_in", [B, 2 * LK], F32)
    ag_in2 = nc.dram_tensor("ag_in2", [256, 24], F32)
    # 3 collectives: A=qt0-7 (fires mid-run), B=qt8-13 (fires after qt13 so
    # the CC ring is free when the tail needs it), C=qt14+qt15 (fires right
    # after qt15's candidates land; phase 2 serves qt14 on pid 6 and qt15 on
    # pid 7). Pads keep the single-DMA affine reads (base f(pid), stride
    # c->rows-per-core) in range for every pid.
    ag_A = nc.dram_tensor("ag_A", [NCORES * 1024, 2 * LK], F32,
                          addr_space="Shared")
    ag_B = nc.dram_tensor("ag_B", [NCORES * 768 + 896, 2 * LK], F32,
                          addr_space="Shared")
    ag_C = nc.dram_tensor("ag_C", [896 + NCORES * 256 + 128, 24], F32,
                          addr_space="Shared")

    with tile.TileContext(nc) as tc:
        with tc.tile_pool(name="res", bufs=1) as res, \
             tc.tile_pool(name="zps", bufs=7, space="PSUM") as zps, \
             tc.tile_pool(name="zrowp", bufs=1) as zrowp, \
             tc.tile_pool(name="small", bufs=3) as small:

            # ---- resident tiles (DMA-filled) ----
            xt = [res.tile([128, B], F32R, name=f"xt{k}") for k in range(4)]
            yt = [[res.tile([128, CHUNKS[c]], F32R, name=f"yt{k}_{c}")
                   for c in range(NCHUNK)] for k in range(4)]
            xn_all = res.tile([128, QTILES], F32)
            bndt_f = res.tile([128, K * VW], F32)
            yn3 = res.tile([4, COLS], BF16)
            ones4 = res.tile([4, 128], BF16)

            nc.vector.memset(ones4[:], 0.0)
            nc.vector.memset(ones4[0:3, :], -1.0)
            nc.sync.dma_start(yn3[:], yn3_in[:])

            # critical path first: y chunk 0 + the x columns for qtiles 0-3,
            # then the rest of x/y; ynb/bndt/xn are not needed until the
            # first drains/phases complete, so they go last.
            def _load_y(c):
                co = _coff(c)
                for k in range(4):
                    nc.sync.dma_start(yt[k][c][:],
                                      y_in[k][:, co:co + CHUNKS[c]])
            _load_y(0)
            for k in range(4):
                nc.sync.dma_start(xt[k][:, 0:512], x_in[k][:, 0:512])
            _load_y(1)
            for k in range(4):
                nc.sync.dma_start(xt[k][:, 512:B], x_in[k][:, 512:B])
            for c in range(2, NCHUNK):
                _load_y(c)
            nc.sync.dma_start(xn_all[:], xn_in[:])
            nc.sync.dma_start(bndt_f[:], bndt_in[:])

            pid_sp = nc.sync.partition_id()
            agA3 = ag_A[:].rearrange("(c r) w -> c r w", c=NCORES)

            def do_qtile(qt):
                qs = qt * 128
                zh = [zrowp.tile([128, H0], F32, name=f"zh0_{qt}", tag="zh0"),
                      zrowp.tile([128, H1], F32, name=f"zh1_{qt}", tag="zh1")]
                for c in range(NCHUNK):
                    cw = CHUNKS[c]
                    co = _coff(c)
                    ps = zps.tile([128, 512], F32)
                    nc.tensor.matmul(ps[:, :cw], ones4[:], yn3[:, co:co + cw],
                                     start=True, stop=False)
                    for k in range(4):
                        nc.tensor.matmul(ps[:, :cw],
                                         xt[k][:, qs:qs + 128],
                                         yt[k][c][:, :cw],
                                         start=False, stop=(k == 3))
                    if c < HSPLIT:
                        nc.scalar.copy(zh[0][:, co:co + cw], ps[:, :cw])
                    else:
                        nc.scalar.copy(zh[1][:, co - H0:co - H0 + cw],
                                       ps[:, :cw])

                cv = small.tile([128, LK], F32, name=f"cv{qt}", tag="cv")
                ci = small.tile([128, LK], U16, name=f"ci{qt}", tag="ci")
                gf = small.tile([128, LK], F32, name=f"gf{qt}", tag="gf")
                for h in range(2):
                    s = slice(h * 8, h * 8 + 8)
                    nc.vector.max(cv[:, s], zh[h][:])
                    nc.vector.max_index(ci[:, s], cv[:, s], zh[h][:])
                nc.vector.tensor_copy(gf[:], ci[:])
                nc.vector.tensor_scalar(out=gf[:, 8:16], in0=gf[:, 8:16],
                                        scalar1=float(H0), scalar2=None,
                                        op0=AL.add)
                if qt < 14:
                    nc.sync.dma_start(ag_in[qs:qs + 128, 0:LK], cv[:])
                    nc.sync.dma_start(ag_in[qs:qs + 128, LK:2 * LK], gf[:])
                else:
                    gfu = small.tile([128, LK], U16, name=f"gfu{qt}",
                                     tag="gfu")
                    nc.vector.tensor_copy(gfu[:], gf[:])
                    r2 = (qt - 14) * 128
                    nc.sync.dma_start(ag_in2[r2:r2 + 128, 0:LK], cv[:])
                    nc.sync.dma_start(
                        ag_in2[r2:r2 + 128, LK:LK + 8],
                        gfu[:].bitcast(F32))

            def do_collective(t, lo, hi, pad):
                nc.gpsimd.collective_compute(
                    "AllGather", AL.bypass,
                    replica_groups=[list(range(NCORES))],
                    ins=[ag_in[lo:hi, :].opt()],
                    outs=[t[pad:pad + NCORES * (hi - lo), :].opt()])

            def do_phase(l):
                # l=0: qt=pid from A; l=1: qt=pid+8 from B (junk for pid 7);
                # l=2: qt=15 from C (real for pid 7 only).
                gw = 24 if l == 2 else 2 * LK
                gvl = small.tile([128, NCORES, gw], F32,
                                 name=f"gvl{l}", tag="gvl", bufs=2)
                if l == 0:
                    src = agA3[:, bass.ds(pid_sp * 128, 128), :]
                elif l == 1:
                    src = (ag_B[bass.ds(pid_sp * 128, NCORES * 768), :]
                           .rearrange("(c r) w -> c r w", c=NCORES)
                           [:, 0:128, :])
                else:
                    # qt14 rows for pid 6 (base 896), qt15 for pid 7 (1024);
                    # pids 0-5 read the junk front pad (output unharvested)
                    src = (ag_C[bass.ds(128 + pid_sp * 128, NCORES * 256), :]
                           .rearrange("(c r) w -> c r w", c=NCORES)
                           [:, 0:128, :])
                nc.sync.dma_start(gvl[:], src.rearrange("c p w -> p c w"))
                gv = gvl[:, :, 0:LK]
                if l == 2:
                    gl = (gvl[:, :, LK:LK + 8].bitcast(U16))
                else:
                    gl = gvl[:, :, LK:2 * LK]

                # exact top-16 of the 128 candidates + payload compaction
                t8a = small.tile([128, 8], F32, name=f"t8a{l}", tag="t8a", bufs=2)
                t8b = small.tile([128, 8], F32, name=f"t8b{l}", tag="t8b", bufs=2)
                m1 = small.tile([128, NG], F32, name=f"m1{l}", tag="m1", bufs=2)
                m2 = small.tile([128, NG], F32, name=f"m2{l}", tag="m2", bufs=2)
                nc.vector.max(t8a[:], gv)
                nc.vector.match_replace(m1[:], t8a[:], gv, NEG)
                nc.vector.max(t8b[:], m1[:])
                nc.vector.match_replace(m2[:], t8b[:], m1[:], NEG)
                mask = small.tile([128, NG], F32, name=f"mk{l}", tag="mk", bufs=2)
                nc.vector.tensor_scalar(out=mask[:], in0=m2[:],
                                        scalar1=-2e38, scalar2=None,
                                        op0=AL.is_le)
                cs = small.tile([128, NG], F32, name=f"cs{l}", tag="cs", bufs=2)
                nc.vector.tensor_tensor_scan(out=cs[:], data0=mask[:],
                                             data1=mask[:], initial=0.0,
                                             op0=AL.add, op1=AL.bypass)
                rk = small.tile([128, NG], F32, name=f"rk{l}", tag="rk", bufs=2)
                nc.vector.tensor_tensor(out=rk[:], in0=cs[:],
                                        in1=mask[:], op=AL.mult)
                nc.vector.tensor_scalar(out=rk[:], in0=rk[:], scalar1=-1.0,
                                        scalar2=None, op0=AL.add)
                rk16 = small.tile([128, NG], I16, name=f"rk16{l}",
                                  tag="rk16", bufs=2)
                nc.vector.tensor_copy(rk16[:], rk[:])

                vlo = small.tile([128, NG], U16, name=f"vlo{l}", tag="vlo", bufs=2)
                vhi = small.tile([128, NG], U16, name=f"vhi{l}", tag="vhi", bufs=2)
                gvu = gv.bitcast(U16).rearrange("p a (b two) -> p a b two",
                                                two=2)
                nc.vector.tensor_copy(vlo[:], gvu[:, :, :, 0:1])
                nc.vector.tensor_copy(vhi[:], gvu[:, :, :, 1:2])
                glu = small.tile([128, NG], U16, name=f"glu{l}", tag="glu", bufs=2)
                nc.vector.tensor_copy(glu[:], gl)
                slo = small.tile([128, K], U16, name=f"slo{l}", tag="slo", bufs=2)
                shi = small.tile([128, K], U16, name=f"shi{l}", tag="shi", bufs=2)
                sla = small.tile([128, K], U16, name=f"sla{l}", tag="sla", bufs=2)
                for plane, out16 in ((vlo, slo), (vhi, shi), (glu, sla)):
                    nc.gpsimd.local_scatter(
                        out16[:].bitcast(I16), plane[:].bitcast(I16),
                        rk16[:], channels=128, num_elems=K, num_idxs=NG)
                v16 = small.tile([128, K], F32, name=f"v16{l}", tag="v16", bufs=2)
                v16u = v16[:].bitcast(U16).rearrange(
                    "p (a two) -> p a two", two=2)
                nc.vector.tensor_copy(v16u[:, :, 0:1], slo[:])
                nc.vector.tensor_copy(v16u[:, :, 1:2], shi[:])
                gidx = small.tile([128, K], F32, name=f"gx{l}", tag="gx", bufs=2)
                nc.vector.tensor_copy(gidx[:], sla[:])

                xn_col = small.tile([128, 1], F32, name=f"xnc{l}", tag="xnc", bufs=2)
                nc.sync.dma_start(
                    xn_col[:],
                    xn_all[:, bass.ds(pid_sp + (8 if l else 0), 1)])
                dsq = small.tile([128, K], F32, name=f"dsq{l}", tag="dsq", bufs=2)
                nc.scalar.activation(dsq[:], v16[:], AF.Sqrt, scale=-1.0,
                                     bias=xn_col[:, 0:1])
                ew = small.tile([128, K], F32, name=f"ew{l}", tag="ew", bufs=2)
                zsum = small.tile([128, 1], F32, name=f"zs{l}", tag="zs", bufs=2)
                nc.scalar.activation(ew[:], dsq[:], AF.Exp, scale=-1.0,
                                     accum_out=zsum[:, 0:1])
                rz = small.tile([128, 1], F32, name=f"rz{l}", tag="rz", bufs=2)
                nc.vector.reciprocal(rz[:], zsum[:])
                wt = small.tile([128, K], F32, name=f"wt{l}", tag="wt", bufs=2)
                nc.vector.tensor_scalar(out=wt[:], in0=ew[:],
                                        scalar1=rz[:, 0:1], scalar2=None,
                                        op0=AL.mult)
                # batched votes: S[q, r, c] = w_r * [bnd_c <= idx_r],
                # tree-reduced over r; vote_c = S_c - S_{c+1}.
                bnd3 = bndt_f[:].rearrange("p (r c) -> p r c", r=K)
                gidx_b = gidx[:].unsqueeze(2).broadcast_to([128, K, VW])
                wt_b = wt[:].unsqueeze(2).broadcast_to([128, K, VW])
                sm = small.tile([128, K, VW], F32, name=f"sm{l}", tag="sm", bufs=1)
                nc.vector.tensor_tensor(out=sm[:], in0=bnd3, in1=gidx_b,
                                        op=AL.is_le)
                nc.vector.tensor_tensor(out=sm[:], in0=sm[:], in1=wt_b,
                                        op=AL.mult)
                for half in (8, 4, 2, 1):
                    nc.vector.tensor_tensor(
                        out=sm[:, 0:half, :], in0=sm[:, 0:half, :],
                        in1=sm[:, half:2 * half, :], op=AL.add)
                vote = small.tile([128, NCLASS], F32, name=f"vote{l}",
                                  tag="vote", bufs=2)
                nc.vector.tensor_tensor(out=vote[:],
                                        in0=sm[:, 0, 0:NCLASS],
                                        in1=sm[:, 0, 1:VW],
                                        op=AL.subtract)
                nc.sync.dma_start(out_d[l * 128:(l + 1) * 128, :], vote[:])

            for qt in range(8):
                do_qtile(qt)
            do_collective(ag_A, 0, 1024, 0)
            do_qtile(8)
            do_phase(0)
            for qt in range(9, 14):
                do_qtile(qt)
            do_collective(ag_B, 1024, 1792, 0)
            do_qtile(14)
            do_qtile(15)
            do_phase(1)
            nc.gpsimd.collective_compute(
                "AllGather", AL.bypass,
                replica_groups=[list(range(NCORES))],
                ins=[ag_in2[:, :].opt()],
                outs=[ag_C[896:896 + NCORES * 256, :].opt()])
            do_phase(2)

    nc.finalize()
    return nc


def _bf16(a):
    """Round fp32 -> bf16 (round-to-nearest-even), keep fp32 container."""
    u = a.view(np.uint32)
    rounded = (u.astype(np.uint64) + 0x7FFF
               + ((u >> 16) & 1)) >> 16
    return (rounded.astype(np.uint32) << 16).view(np.float32)


def _f32r(a):
    i = a.view(np.uint32).astype(np.int64)
    i = (i + 0x400) & ~0x7FF
    return (i & 0xFFFFFFFF).astype(np.uint32).view(np.float32)


def _bf16_bits(a):
    """fp32 -> bf16 (ml_dtypes.bfloat16 array, round-to-nearest-even)."""
    u = a.view(np.uint32)
    rounded = (u.astype(np.uint64) + 0x7FFF + ((u >> 16) & 1)) >> 16
    return rounded.astype(np.uint16).view(ml_dtypes.bfloat16)


def _host_prep(x, tf, tl):
    """Produce per-core input dicts."""
    x = np.ascontiguousarray(np.asarray(x, np.float32))
    tf = np.ascontiguousarray(np.asarray(tf, np.float32))
    tl = np.asarray(tl, np.int64)

    X = 2.0 * x                                    # fold z = X.y - yn
    xn = np.sum(x.astype(np.float64) * x, axis=1).astype(np.float32)
    xn_all = xn.reshape(QTILES, 128).T.copy()      # [128, 16]

    xparts = [_f32r(X)]

    # equal-boundary deal: global sort by label, class c dealt round-robin
    # to cores, padded so every core has identical class boundaries.
    perm = np.argsort(tl, kind="stable")
    tf_s = tf[perm]
    counts = np.bincount(tl, minlength=NCLASS)
    m = (counts + NCORES - 1) // NCORES            # per-core padded counts
    assert m.sum() <= COLS, m.sum()
    bnd = np.concatenate([[0], np.cumsum(m)[:-1]])
    bnd_ext = np.concatenate([bnd.astype(np.float32) - 0.5, [1.0e9]])
    bndt = np.tile(bnd_ext.astype(np.float32), K)
    bndt_b = np.broadcast_to(bndt, (128, K * (NCLASS + 1))).copy()
    gstart = np.concatenate([[0], np.cumsum(counts)[:-1]])

    core_feats = np.zeros((NCORES, COLS, D), np.float32)
    core_valid = np.zeros((NCORES, COLS), bool)
    for cls in range(NCLASS):
        rows = tf_s[gstart[cls]:gstart[cls] + counts[cls]]
        for c in range(NCORES):
            part = rows[c::NCORES]
            core_feats[c, bnd[cls]:bnd[cls] + len(part)] = part
            core_valid[c, bnd[cls]:bnd[cls] + len(part)] = True

    in_maps = []
    for c in range(NCORES):
        feats = core_feats[c]
        yn = np.sum(feats.astype(np.float64) * feats,
                    axis=1).astype(np.float32)
        ynp = np.where(core_valid[c], yn, np.float32(1.0e30))
        y1 = _bf16(ynp); r1 = ynp - y1
        y2 = _bf16(r1); r2 = r1 - y2
        y3 = _bf16(r2)
        yn3 = np.zeros((4, COLS), ml_dtypes.bfloat16)
        yn3[0] = _bf16_bits(y1)
        yn3[1] = _bf16_bits(y2)
        yn3[2] = _bf16_bits(y3)

        yparts = [_f32r(feats)]

        mm = {
            "yn3": yn3,
            "xn": xn_all,
            "bndt": bndt_b,
        }
        for p, xp in enumerate(xparts):
            # [2048, 512] -> k blocks [128, 2048]
            for k in range(4):
                blk = np.ascontiguousarray(xp[:, k * 128:(k + 1) * 128].T)
                mm[f"x{p}_{k}"] = blk
        for p, yp in enumerate(yparts):
            for k in range(4):
                blk = np.ascontiguousarray(yp[:, k * 128:(k + 1) * 128].T)
                mm[f"y{p}_{k}"] = blk
        in_maps.append(mm)
    return in_maps


_NC_CACHE = {}
LAST_RESULTS = None


def kernel(x, train_features, train_labels, **run_kwargs):
    global LAST_RESULTS
    in_maps = _host_prep(x, train_features, train_labels)
    if "v3" not in _NC_CACHE:
        _NC_CACHE["v3"] = build()
    res = bass_utils.run_bass_kernel_spmd(
        _NC_CACHE["v3"], in_maps, core_ids=list(range(NCORES)),
        **run_kwargs)
    LAST_RESULTS = res
    out = np.zeros((B, NCLASS), np.float32)
    for c in range(NCORES):
        o = res.results[c]["out"]
        out[c * 128:(c + 1) * 128] = o[0:128]
        blk = 1 if c < 6 else 2
        out[(8 + c) * 128:(9 + c) * 128] = o[blk * 128:(blk + 1) * 128]
    return out
